# revision 23
# baseline (speedup 1.0000x reference)
"""Trainium2 Bass kernel for AdaptivePrototypeContrastiveLoss.

Strategy
--------
Host (cheap, O(N*D) bookkeeping):
  * closed-form momentum EMA + LAPACK QR -> new prototypes  [7,256]
  * row-normalize feats, stable-sort rows by label; scatter the sorted
    columns into 7 uniform zero-padded class segments (width = max class
    count rounded up to 16) and append 7 per-class sum columns (Y) so
    the tiny "positive" matmul rides the main sweep
  * the whole scalar epilogue (log, threshold, mean) runs on host from
    shipped per-(row,class) exp sums - it is O(N) numpy work

Device (8 NeuronCores, SPMD, no collectives; all O(N^2) work):
  * row-shard: each core owns 8 row-tiles of 128 rows (64 tiles cover
    rows 0..8191); the last 7 rows' column sweep (row-tile 64) is
    split column-wise across all 8 cores as class-pure 512-chunks
  * per row-tile: G = rows @ feats^T via PE (fp8-e4m3 DoubleRow, f32
    PSUM, K=256 per instruction, 512-col chunks in 2048-col supertiles)
  * ACT computes exp(A*sim + BIAS) from PSUM into a flat bf16 SBUF
    scratch row (2048-wide ACTs + a short tail)
  * DVE reduces each class segment with a log2 fold tree: 4 batched
    bf16 tensor_tensor adds on a [128,7,*] strided view (2x DVE mode),
    then one batched TENSOR_REDUCE -> [128,7] class sums.  Uniform
    segment widths keep every AP identical across cores (SPMD-uniform).
  * input DMA is split fine-grained and ordered so the first row-tile's
    operands land first; compute starts early and the rest streams in
  * the global max subtraction is replaced by the constant M0=12.5 (the
    max only enters through ~1e-8-scale eps terms, verified offline)
  * per-core output: [128, NT*8] class sums + [128, NT*8] raw Y dots
    + 3 column-chunk exp sums for the shared row-tile 64, one DMA
Host: per-row loss, threshold, mean over 8x[128,136] partials.
"""

import ml_dtypes
import numpy as np

import concourse.bass as bass
import concourse.tile as tile
from concourse import mybir
from concourse.bass_utils import run_bass_kernel_spmd

# ---- problem constants (hardcoded per spec) ----
TEMP = 0.08
EPS = 1e-8
GAMMA = 0.99
BETA = 0.5 * (1.0 - GAMMA)
B, D, C = 8192, 256, 7
N = B + C                      # 8199 rows/cols of the score matrix
NCORES = 8
NT = 8                         # full row-tiles per core (8*8*128 = 8192)
ROWS_PER_CORE = NT * 128       # 1024
W = 1280                       # padded class-segment width (adaptive, <=1280)
SUPER = 2048                   # psum supertile width (4 banks)
T8W = 1536                     # per-core share of row-tile 64's columns
OUTW = 136                     # out: 64 slots | 64 praw | 3 t8 | pad
M0 = 12.5                      # constant stand-in for the global max
A_SCALE = 0.5 / float(np.float32(TEMP))
BIAS = (0.5 + EPS) / float(np.float32(TEMP)) - M0
PAD_EXP = float(ml_dtypes.bfloat16(np.exp(np.float32(BIAS))))

F32 = mybir.dt.float32
BF16 = mybir.dt.bfloat16
FP8 = mybir.dt.float8e4
FP8NP = mybir.dt.np(mybir.dt.float8e4)
ALU = mybir.AluOpType
ACTF = mybir.ActivationFunctionType


def _geometry(w_seg):
    """Derived column-layout geometry for a given padded segment width."""
    npad = C * w_seg
    nsup = npad // SUPER
    tailw = npad - nsup * SUPER
    assert 0 < tailw <= SUPER
    tail_chunks = []
    o = 0
    while o < tailw:
        wch = min(512, tailw - o)
        tail_chunks.append((o, wch))
        o += wch
    if tail_chunks and tail_chunks[-1][1] + 8 <= 512:
        ol, wl = tail_chunks[-1]
        tail_chunks[-1] = (ol, wl + 8)
    else:
        tail_chunks.append((tailw, 8))
    return {
        "W": w_seg, "NPAD": npad, "YOFF": npad, "NF": npad + 16,
        "NSUP": nsup, "TAILW": tailw, "TAIL_CHUNKS": tail_chunks,
    }


def _prune_redundant_waits(nc):
    """Transitive reduction of semaphore waits via vector clocks.

    All data sems are monotonic counting sems (sem-inc / sem-ge-imm).
    Soundness rules:
      * an instruction starts only after all its waits are satisfied AND
        all earlier same-queue instructions' waits were satisfied (the
        queue stalls at the head), so VC_start inherits prev-same-engine
        VC_start plus prev's wait implications -- but NOT prev's own
        updates (pipelined overlap);
      * a satisfied wait (sem >= v) implies the producer instruction
        completed, which (in-order retirement) implies every earlier
        instruction on the producer's engine completed, so it yields
        that engine's update prefix plus the producer's VC_start.
    Waits with non-inc sems or unknown producers are kept verbatim.
    """
    insts = []
    for func in nc.m.functions:
        for blk in func.blocks:
            insts.extend(blk.instructions)

    bad_ids = set()
    cum = {}
    producer = {}       # sem id -> list of (cum_after, inst_idx)
    eng_pos = {}        # inst idx -> (engine, idx_on_engine)
    eng_insts = {}      # engine -> [inst idx]
    eng_prefix = {}     # inst idx -> dict sem->cum (this engine's updates
                        # up to and including this instruction)
    run_prefix = {}     # engine -> running dict
    for idx, inst in enumerate(insts):
        e = str(inst.engine)
        lst = eng_insts.setdefault(e, [])
        eng_pos[idx] = (e, len(lst))
        lst.append(idx)
        si = getattr(inst, "sync_info", None)
        is_dma = type(inst).__name__ == "InstDMACopy"
        pre = run_prefix.setdefault(e, {})
        if si and si.on_update:
            for u in si.on_update:
                if u.sync_type != "semaphore":
                    continue
                if u.update_mode != "sem-inc" or is_dma:
                    bad_ids.add(u.id)
                    continue
                cum[u.id] = cum.get(u.id, 0) + (u.update_value or 1)
                producer.setdefault(u.id, []).append((cum[u.id], idx))
                pre = dict(pre)
                pre[u.id] = cum[u.id]
                run_prefix[e] = pre
        eng_prefix[idx] = pre

    def vc_merge(a, b):
        if not b:
            return a
        out = dict(a)
        for k, v in b.items():
            if out.get(k, -1) < v:
                out[k] = v
        return out

    def wait_implications(w, idx):
        """VC facts guaranteed once wait w (held by inst idx) is satisfied."""
        facts = {}
        if w.sync_type != "semaphore" or w.wait_mode != "sem-ge-imm":
            return facts
        v = w.wait_value or 0
        facts[w.id] = v
        if w.id in bad_ids or w.id not in producer:
            return facts
        p_idx = None
        for cv, pi in producer[w.id]:
            if cv >= v:
                p_idx = pi
                break
        if p_idx is None or p_idx >= idx:
            return facts
        facts = vc_merge(facts, eng_prefix[p_idx])
        facts = vc_merge(facts, vc_start[p_idx])
        return facts

    vc_start = [None] * len(insts)
    kept_counts = 0
    dropped = 0
    for idx, inst in enumerate(insts):
        e, k = eng_pos[idx]
        if k == 0:
            base = {}
        else:
            prev = eng_insts[e][k - 1]
            base = vc_start[prev]
            psi = getattr(insts[prev], "sync_info", None)
            if psi and psi.on_wait:
                for w in psi.on_wait:
                    base = vc_merge(base, wait_implications(w, prev))
        si = getattr(inst, "sync_info", None)
        waits = list(si.on_wait) if si and si.on_wait else []
        if len(waits) > 1:
            impls = [wait_implications(w, idx) for w in waits]
            keep = list(range(len(waits)))
            changed = True
            while changed:
                changed = False
                for j in list(keep):
                    w = waits[j]
                    if w.sync_type != "semaphore" or w.wait_mode != "sem-ge-imm":
                        continue
                    ctx = dict(base)
                    for j2 in keep:
                        if j2 != j:
                            ctx = vc_merge(ctx, impls[j2])
                    if ctx.get(w.id, -1) >= (w.wait_value or 0):
                        keep.remove(j)
                        changed = True
                        break
            if len(keep) < len(waits):
                dropped += len(waits) - len(keep)
                si.on_wait = [waits[j] for j in keep]
                waits = list(si.on_wait)
        kept_counts += len(waits)
        vc_start[idx] = base
    return dropped


def _split_multi_waits(nc):
    """This container's walrus accepts only ONE sync wait per instruction;
    split extra waits into standalone single-wait EventSemaphore insts."""
    n_new = 0
    for func in nc.m.functions:
        for blk in func.blocks:
            new_insts = []
            for inst in blk.instructions:
                si = getattr(inst, "sync_info", None)
                waits = list(si.on_wait) if si and si.on_wait else []
                if len(waits) > 1:
                    for i, w in enumerate(waits[:-1]):
                        n_new += 1
                        ev = mybir.InstEventSemaphore(
                            name=f"{inst.name}-wsplit{i}",
                            engine=inst.engine,
                            ins=[],
                            outs=[],
                            sync_info=mybir.SyncInfo(on_wait=[w], on_update=[]),
                            bass_nofuse=True,
                        )
                        new_insts.append(ev)
                    si.on_wait = [waits[-1]]
                new_insts.append(inst)
            blk.instructions = new_insts
    return n_new


def _host_prep(features, labels, prototypes, momentums):
    features = np.asarray(features, dtype=np.float32)
    labels = np.asarray(labels).astype(np.int64)
    prototypes = np.asarray(prototypes, dtype=np.float32)
    momentums = np.asarray(momentums, dtype=np.float32)

    # ---- prototype update: closed form of the sequential EMA scan ----
    counts_feat = np.bincount(labels, minlength=C)
    rank = np.zeros(B, dtype=np.int64)
    seen = np.zeros(C, dtype=np.int64)
    for i, l in enumerate(labels):
        rank[i] = seen[l]
        seen[l] += 1
    w = BETA * (GAMMA ** (counts_feat[labels] - 1 - rank).astype(np.float64))
    S = np.zeros((C, B))
    S[labels, np.arange(B)] = w
    m_final = S @ features.astype(np.float64)
    wsum = np.bincount(labels, weights=w, minlength=C)
    m_final -= wsum[:, None] * prototypes.astype(np.float64)
    m_final += (GAMMA ** counts_feat.astype(np.float64))[:, None] * momentums.astype(
        np.float64
    )
    target = prototypes.astype(np.float64) + m_final
    q, _ = np.linalg.qr(target.T.astype(np.float32))
    new_protos = q.T.astype(np.float32)

    # ---- normalized, label-sorted gram operands ----
    feats = np.concatenate([features, new_protos], 0)
    labs = np.concatenate([labels, np.arange(C, dtype=np.int64)])
    nrm = np.linalg.norm(feats.astype(np.float64), axis=-1)
    fhat = feats.astype(np.float64) / nrm[:, None]
    perm = np.argsort(labs, kind="stable")
    fs = fhat[perm]
    ls = labs[perm]
    counts_all = np.bincount(ls, minlength=C)          # includes protos
    bounds = np.concatenate([[0], np.cumsum(counts_all)])  # class col ranges
    w_seg = int(-(-int(counts_all.max()) // 16) * 16)  # round up to mult of 16
    w_seg = max(w_seg, 512 + 16)  # keep >=2 chunks per class segment
    geom = _geometry(w_seg)
    pads = w_seg - counts_all                          # zero-pad cols per class

    fs32 = fs.astype(np.float32)
    Y = np.zeros((D, 8), dtype=np.float64)
    for c in range(C):
        Y[:, c] = fs[bounds[c]:bounds[c + 1]].sum(0)

    # columns scattered into uniform zero-padded class segments
    W, NPAD, YOFF, NF = geom["W"], geom["NPAD"], geom["YOFF"], geom["NF"]
    ftcols = np.zeros((NF, D), dtype=np.float32)
    for c in range(C):
        cnt = int(counts_all[c])
        ftcols[c * W:c * W + cnt] = fs32[bounds[c]:bounds[c + 1]]
    ftcols[YOFF:YOFF + 8] = Y.T.astype(np.float32)
    ft = np.ascontiguousarray(
        ftcols.T.reshape(2, 128, NF).transpose(1, 0, 2)
    ).astype(FP8NP)  # [partition, k-half, col] for DoubleRow

    # ---- shared row-tile 64 (last 7 rows), column-split across cores ----
    t8block = np.zeros((128, D), dtype=np.float32)
    t8block[:N - B] = fs32[B:N]
    t8rows = np.ascontiguousarray(
        t8block.T.reshape(2, 128, 128).transpose(1, 0, 2)
    ).astype(FP8NP)
    chunk_cols = []  # class-pure 512-col chunks (padded-layout col indices)

    chunk_cls = []
    for c in range(C):
        cols = np.arange(c * W, c * W + int(counts_all[c]))
        for o in range(0, len(cols), 512):
            chunk_cols.append(cols[o:o + 512])
            chunk_cls.append(c)
    n_cpc = T8W // 512  # chunks per core
    assert len(chunk_cols) <= NCORES * n_cpc, len(chunk_cols)
    while len(chunk_cols) < NCORES * n_cpc:
        chunk_cols.append(np.zeros(0, dtype=np.int64))
        chunk_cls.append(-1)
    t8meta = []  # (class, n_pad) per chunk for the host-side combine
    ft_np = np.asarray(ft)
    t8cols_per_core = []
    for core in range(NCORES):
        arr = np.zeros((128, 2, T8W), dtype=FP8NP)
        for j in range(n_cpc):
            ci = core * n_cpc + j
            cols = chunk_cols[ci]
            arr[:, :, j * 512:j * 512 + len(cols)] = ft_np[:, :, cols]
            t8meta.append((chunk_cls[ci], 512 - len(cols)))
        t8cols_per_core.append(arr)

    per_core = []
    for core in range(NCORES):
        base = core * ROWS_PER_CORE
        rows_kt = np.ascontiguousarray(
            fs32[base:base + ROWS_PER_CORE].T.reshape(2, 128, ROWS_PER_CORE)
            .transpose(1, 0, 2)
        ).astype(FP8NP)
        per_core.append(
            {
                "ft": ft,
                "rows": rows_kt,
                "t8rows": t8rows,
                "t8cols": t8cols_per_core[core],
            }
        )

    cnt = counts_all[ls] - 1
    selfsim = (fs32.astype(np.float64) ** 2).sum(1)
    crosspad_all = (pads.sum() - pads[ls]) * PAD_EXP
    host = {
        "ls": ls, "counts_all": counts_all, "fs": fs, "Y": Y,
        "t8meta": t8meta, "selfsim": selfsim, "cnt": cnt,
        "crosspad": crosspad_all,
    }
    return per_core, host, geom


def _build_graph(geom):
    W, NPAD, YOFF, NF = geom["W"], geom["NPAD"], geom["YOFF"], geom["NF"]
    NSUP, TAILW = geom["NSUP"], geom["TAILW"]
    nc = bass.Bass()
    ft_d = nc.declare_dram_parameter("ft", [128, 2, NF], FP8, isOutput=False)
    rows_d = nc.declare_dram_parameter(
        "rows", [128, 2, ROWS_PER_CORE], FP8, isOutput=False
    )
    t8r_d = nc.declare_dram_parameter("t8rows", [128, 2, 128], FP8, isOutput=False)
    t8c_d = nc.declare_dram_parameter("t8cols", [128, 2, T8W], FP8, isOutput=False)
    out_d = nc.declare_dram_parameter("out", [128, OUTW], F32, isOutput=True)

    with tile.TileContext(nc) as tc:
        with (
            tc.tile_pool(name="persist", bufs=1) as persist,
            tc.tile_pool(name="ps", bufs=2, space="PSUM") as psA,
        ):
            # --- resident inputs; DMA ordered so tile 0 operands land first
            rows_sb = persist.tile([128, 2, ROWS_PER_CORE], FP8, tag="rows")
            ft_sb = persist.tile([128, 2, NF], FP8, tag="ft")
            nc.sync.dma_start(out=rows_sb[:, :, 0:128], in_=rows_d[:, :, 0:128])
            nc.sync.dma_start(
                out=ft_sb[:, :, NSUP * SUPER:NF], in_=ft_d[:, :, NSUP * SUPER:NF]
            )
            nc.sync.dma_start(out=ft_sb[:, :, 0:512], in_=ft_d[:, :, 0:512])
            for o in range(512, 2048, 512):
                nc.sync.dma_start(
                    out=ft_sb[:, :, o:o + 512], in_=ft_d[:, :, o:o + 512]
                )
            nc.sync.dma_start(out=rows_sb[:, :, 128:1024], in_=rows_d[:, :, 128:1024])
            for o in range(2048, NSUP * SUPER, 2048):
                hi = min(o + 2048, NSUP * SUPER)
                nc.sync.dma_start(out=ft_sb[:, :, o:hi], in_=ft_d[:, :, o:hi])
            t8r_sb = persist.tile([128, 2, 128], FP8, tag="t8r")
            nc.sync.dma_start(out=t8r_sb[:], in_=t8r_d[:])
            t8c_sb = persist.tile([128, 2, T8W], FP8, tag="t8c")
            nc.sync.dma_start(out=t8c_sb[:], in_=t8c_d[:])

            bias_exp = persist.tile([128, 1], F32, tag="bias_exp")
            nc.vector.memset(bias_exp[:], float(BIAS))
            outbuf = persist.tile([128, OUTW], F32, tag="outbuf")
            nc.vector.memset(outbuf[:], 0.0)

            scr_a = persist.tile([128, NPAD], BF16, tag="scr0")
            scr_b = persist.tile([128, NPAD], BF16, tag="scr1")
            scr_c = persist.tile([128, NPAD], BF16, tag="scr2")
            scrs = [scr_a, scr_b, scr_c]
            f1 = persist.tile([128, C, W // 2], BF16, tag="f1")
            f2 = persist.tile([128, C, W // 4], BF16, tag="f2")
            f3 = persist.tile([128, C, W // 8], BF16, tag="f3")
            f4 = persist.tile([128, C, W // 16], BF16, tag="f4")
            scr8 = persist.tile([128, T8W], BF16, tag="scr8")

            # --- main loop over row-tiles ---
            for t in range(NT):
                scr = scrs[t % 3]
                lhs_t = rows_sb[:, :, t * 128:(t + 1) * 128]
                # tail first: keeps ACT fed across tile boundaries (PE stays
                # a full supertile ahead when s3's ACT completes)
                pst = psA.tile([128, SUPER], F32, tag="ps")
                for o, wch in geom["TAIL_CHUNKS"]:
                    nc.tensor.matmul(
                        pst[:, o:o + wch],
                        lhsT=lhs_t,
                        rhs=ft_sb[:, :, NSUP * SUPER + o:NSUP * SUPER + o + wch],
                        start=True,
                        stop=True,
                        perf_mode=mybir.MatmulPerfMode.DoubleRow,
                    )
                nc.scalar.activation(
                    scr[:, NSUP * SUPER:NPAD],
                    pst[:, 0:TAILW],
                    ACTF.Exp,
                    bias=bias_exp[:],
                    scale=float(A_SCALE),
                )
                nc.vector.tensor_copy(
                    outbuf[:, 64 + t * 8:64 + t * 8 + 8],
                    pst[:, TAILW:TAILW + 8],
                )
                for s in range(NSUP):
                    lo = s * SUPER
                    ps = psA.tile([128, SUPER], F32, tag="ps")
                    for j in range(SUPER // 512):
                        nc.tensor.matmul(
                            ps[:, j * 512:(j + 1) * 512],
                            lhsT=lhs_t,
                            rhs=ft_sb[:, :, lo + j * 512:lo + (j + 1) * 512],
                            start=True,
                            stop=True,
                            perf_mode=mybir.MatmulPerfMode.DoubleRow,
                        )
                    nc.scalar.activation(
                        scr[:, lo:lo + SUPER],
                        ps[:],
                        ACTF.Exp,
                        bias=bias_exp[:],
                        scale=float(A_SCALE),
                    )
                # class-segment sums: batched bf16 fold tree + one reduce
                s3 = scr[:].rearrange("p (c w) -> p c w", c=C)
                h = W // 2
                nc.vector.tensor_tensor(
                    out=f1[:], in0=s3[:, :, 0:h], in1=s3[:, :, h:W],
                    op=ALU.add,
                )
                nc.vector.tensor_tensor(
                    out=f2[:], in0=f1[:, :, 0:h // 2], in1=f1[:, :, h // 2:h],
                    op=ALU.add,
                )
                nc.vector.tensor_tensor(
                    out=f3[:], in0=f2[:, :, 0:h // 4], in1=f2[:, :, h // 4:h // 2],
                    op=ALU.add,
                )
                nc.vector.tensor_tensor(
                    out=f4[:], in0=f3[:, :, 0:h // 8], in1=f3[:, :, h // 8:h // 4],
                    op=ALU.add,
                )
                nc.vector.reduce_sum(
                    outbuf[:, t * 8:t * 8 + C], f4[:], mybir.AxisListType.X
                )

                if t == 3:
                    # shared row-tile 64 (this core's column slice), placed
                    # mid-loop so it fills pipeline gaps instead of a tail
                    ps8 = psA.tile([128, SUPER], F32, tag="ps")
                    for j in range(T8W // 512):
                        nc.tensor.matmul(
                            ps8[:, j * 512:(j + 1) * 512],
                            lhsT=t8r_sb[:],
                            rhs=t8c_sb[:, :, j * 512:(j + 1) * 512],
                            start=True,
                            stop=True,
                            perf_mode=mybir.MatmulPerfMode.DoubleRow,
                        )
                    nc.scalar.activation(
                        scr8[:], ps8[:, 0:T8W], ACTF.Exp,
                        bias=bias_exp[:], scale=float(A_SCALE),
                    )
                    s83 = scr8[:].rearrange("p (c w) -> p c w", c=T8W // 512)
                    nc.vector.reduce_sum(
                        outbuf[:, 128:131], s83, mybir.AxisListType.X
                    )

            nc.sync.dma_start(out=out_d[:], in_=outbuf[:])
    return nc


def _combine(results, host):
    """Host-side epilogue: per-row loss from shipped class sums."""
    ls = host["ls"]
    fs, Y = host["fs"], host["Y"]
    selfsim, cnt, crosspad = host["selfsim"], host["cnt"], host["crosspad"]

    loss_sum = 0.0
    cnt_sum = 0.0
    for core in range(NCORES):
        o = np.asarray(results[core]["out"], dtype=np.float64)
        slots = o[:, 0:64].reshape(128, NT, 8)     # [p, t, class]
        praw = o[:, 64:128].reshape(128, NT, 8)
        base = core * ROWS_PER_CORE
        g = base + np.arange(NT)[None, :] * 128 + np.arange(128)[:, None]
        own = ls[g]                                 # [p, t]
        stot = slots[:, :, 0:C].sum(-1)
        sown = np.take_along_axis(slots, own[:, :, None], axis=2)[:, :, 0]
        negsum = stot - sown - crosspad[g]
        neg = np.log(negsum + EPS)
        possel = np.take_along_axis(praw, own[:, :, None], axis=2)[:, :, 0]
        pos = (A_SCALE * (possel - selfsim[g]) + BIAS * cnt[g]) / (cnt[g] + EPS)
        loss = neg - pos
        m = loss > 0
        loss_sum += loss[m].sum()
        cnt_sum += m.sum()

    # row-tile 64: rows 8192..8198 — class sums from per-core chunk sums
    n7 = N - B  # 7
    n_cpc = T8W // 512
    classsum = np.zeros((n7, C), dtype=np.float64)
    for core in range(NCORES):
        o = np.asarray(results[core]["out"], dtype=np.float64)
        for j in range(n_cpc):
            cls, n_pad = host["t8meta"][core * n_cpc + j]
            if cls < 0:
                continue
            classsum[:, cls] += o[:n7, 128 + j] - n_pad * PAD_EXP
    stot = classsum.sum(1)
    rows_ls = ls[B:N]
    sown = classsum[np.arange(n7), rows_ls]
    neg = np.log(stot - sown + EPS)
    pos_sel = np.einsum("id,di->i", fs[B:N], Y[:, rows_ls])
    pos = (A_SCALE * (pos_sel - host["selfsim"][B:N]) + BIAS * host["cnt"][B:N]) / (
        host["cnt"][B:N] + EPS
    )
    loss64 = -pos + neg
    m = loss64 > 0
    loss_sum += loss64[m].sum()
    cnt_sum += m.sum()

    val = loss_sum / max(cnt_sum, 1.0) if cnt_sum > 0 else 0.0
    return np.float32(val)


def _run(features, labels, prototypes, momentums, trace=False, trace_kwargs=None):
    per_core, host, geom = _host_prep(features, labels, prototypes, momentums)
    nc = _build_graph(geom)
    _split_multi_waits(nc)
    in_maps = [per_core[i] for i in range(NCORES)]
    kw = {}
    if trace:
        kw = dict(trace=True, trace_cores=list(range(NCORES)))
        if trace_kwargs:
            kw["trace_kwargs"] = trace_kwargs
    res = run_bass_kernel_spmd(nc, in_maps, core_ids=list(range(NCORES)), **kw)
    return _combine(res.results, host), res


def kernel(features, labels, prototypes, momentums):
    val, _ = _run(features, labels, prototypes, momentums)
    return np.array(val, dtype=np.float32)
